# revision 26
# baseline (speedup 1.0000x reference)
"""Multi-head causal attention (RoPE, muP scale) on 8 TRN2 NeuronCores.

Sharding: core c = (b, g) with b = c // 4 (data-parallel batch), g = c % 4
(tensor-parallel head group of 4 heads).  Each core computes q/k/v
projections for its 4 heads, RoPE, causal flash-style attention in the
transposed (sT = [k, q]) orientation, and a partial output projection
o @ wo over its heads.  The host sums the 4 per-group partials per batch
(the tensor-parallel reduce) and stacks the 2 batches.

Matmuls run as float32r (FP22 reads, 1.5 cyc/row) accumulating in fp32
PSUM; softmax runs without max-subtraction (scores are O(0.1) by muP
scaling so exp is well-conditioned), with the causal mask applied as a
0/1 multiply on the diagonal blocks and the denominator accumulated by
an all-ones matmul into a broadcast PSUM tile.
"""

import sys

if "/opt/trn_rl_repo" not in sys.path:
    sys.path.insert(0, "/opt/trn_rl_repo")

import numpy as np

B, T, M, H, D = 2, 2048, 2048, 16, 128
N_CORES = 8
GROUPS = 4
HPG = H // GROUPS          # heads per group (4)
ROTARY_BASE = 10000.0
ATTN_SCALE = 1.0 / 128.0

P = 128                    # partitions
TC = T // 512              # 4 t-chunks of 512
MB = M // P                # 16 m-blocks
TB = T // P                # 16 t-blocks
NQ = 512                   # q-chunk width

_CACHE = {}


def _build_program():
    from concourse import bacc, tile
    import concourse.mybir as mybir

    F32 = mybir.dt.float32
    F32R = mybir.dt.float32r
    AFT = mybir.ActivationFunctionType

    nc = bacc.Bacc("TRN2", target_bir_lowering=False, debug=False,
                   num_devices=N_CORES)

    xt_d = nc.dram_tensor("xt", [M, T], F32R, kind="ExternalInput")
    wq_d = nc.dram_tensor("wq", [P, 2, MB, 256], F32R, kind="ExternalInput")
    wk_d = nc.dram_tensor("wk", [P, 2, MB, 256], F32R, kind="ExternalInput")
    wv_d = nc.dram_tensor("wv", [P, 2, MB, 256], F32R, kind="ExternalInput")
    wo_d = nc.dram_tensor("wo", [P, HPG, M], F32R, kind="ExternalInput")
    cc_d = nc.dram_tensor("trig_cc", [P, T], F32, kind="ExternalInput")
    ss_d = nc.dram_tensor("trig_ss", [P, T], F32, kind="ExternalInput")
    mask_d = nc.dram_tensor("mask01", [P, 4 * NQ], F32, kind="ExternalInput")
    ones_d = nc.dram_tensor("onesw", [P, P], F32R, kind="ExternalInput")
    r_d = nc.dram_tensor("r_out", [T, M], F32, kind="ExternalOutput")

    with tile.TileContext(nc) as tc:
        with (
            tc.tile_pool(name="consts", bufs=1) as consts,
            tc.tile_pool(name="wpool", bufs=1) as wpool,
            tc.tile_pool(name="wopool", bufs=2) as wopool,
            tc.tile_pool(name="qkv", bufs=1) as qkv,
            tc.tile_pool(name="xpool", bufs=6) as xpool,
            tc.tile_pool(name="ppool", bufs=2) as ppool,
            tc.tile_pool(name="rtmp", bufs=2) as rtmp,
            tc.tile_pool(name="opool", bufs=1) as opool,
            tc.tile_pool(name="rout", bufs=2) as rout,
            tc.tile_pool(name="psum", bufs=1, space="PSUM") as psum,
        ):
            cc_sb = consts.tile([P, T], F32, tag="cc")
            nc.sync.dma_start(out=cc_sb[:], in_=cc_d[:])
            ss_sb = consts.tile([P, T], F32, tag="ss")
            nc.sync.dma_start(out=ss_sb[:], in_=ss_d[:])
            mask_sb = consts.tile([P, 4 * NQ], F32, tag="mask")
            nc.sync.dma_start(out=mask_sb[:], in_=mask_d[:])
            ones_sb = consts.tile([P, P], F32R, tag="ones")
            nc.sync.dma_start(out=ones_sb[:], in_=ones_d[:])

            # oT for all 4 heads of the group: [d, h4 * T + t]
            oT_sb = opool.tile([P, HPG * T], F32R, tag="oT")

            xt_tiles = {}

            for pair in range(2):
                wq_sb = wpool.tile([P, MB, 256], F32R, tag="wq", name="wq_sb")
                nc.sync.dma_start(out=wq_sb[:], in_=wq_d[:, pair])
                wk_sb = wpool.tile([P, MB, 256], F32R, tag="wk", name="wk_sb")
                nc.sync.dma_start(out=wk_sb[:], in_=wk_d[:, pair])
                wv_sb = wpool.tile([P, MB, 256], F32R, tag="wv", name="wv_sb")
                nc.sync.dma_start(out=wv_sb[:], in_=wv_d[:, pair])

                qT_sb = [qkv.tile([P, T], F32R, tag=f"qT{hl}", name=f"qT{hl}")
                         for hl in range(2)]
                kT_sb = [qkv.tile([P, T], F32R, tag=f"kT{hl}", name=f"kT{hl}")
                         for hl in range(2)]
                v_sb = qkv.tile([P, TB * 256], F32R, tag="v", name="v_sb")

                # ---- projections + RoPE for this head pair ----
                # xt tile prefetcher: explicit round-robin tags (FIFO slot
                # reuse) and next-chunk DMAs emitted inside the current
                # chunk's tail so the scheduler places them (and their slot
                # waits) early enough to hide the refill at chunk boundaries.
                def ensure_xt(i):
                    if i in xt_tiles:
                        return xt_tiles[i]
                    tcx_i = (i // MB) % TC
                    mb_i = i % MB
                    xt_t = xpool.tile([P, NQ], F32R, bufs=1,
                                      tag=f"xt{i % 8}", name="xt_t")
                    eng = nc.sync if i % 2 == 0 else nc.gpsimd
                    eng.dma_start(
                        out=xt_t[:],
                        in_=xt_d[mb_i * P:(mb_i + 1) * P,
                                 tcx_i * NQ:(tcx_i + 1) * NQ],
                    )
                    xt_tiles[i] = xt_t
                    return xt_t

                for tcx in range(TC):
                    gidx = pair * TC + tcx
                    tsl = slice(tcx * NQ, (tcx + 1) * NQ)
                    psq = [psum.tile([P, NQ], F32, tag=f"q{hl}", name=f"psq{hl}")
                           for hl in range(2)]
                    psk = [psum.tile([P, NQ], F32, tag=f"k{hl}", name=f"psk{hl}")
                           for hl in range(2)]
                    psv = [psum.tile([P, 256], F32, tag=f"v{ts}", name=f"psv{ts}")
                           for ts in range(4)]
                    for mb in range(MB):
                        i = gidx * MB + mb
                        xt_t = ensure_xt(i)
                        xt_tiles.pop(i, None)
                        if mb >= MB - 8 and i + 8 < 2 * TC * MB:
                            ensure_xt(i + 8)
                        st, sp = (mb == 0), (mb == MB - 1)
                        # v first: its PSUM banks free fastest at chunk
                        # boundaries, keeping PE fed while RoPE drains q/k.
                        for ts in range(4):
                            nc.tensor.matmul(
                                psv[ts][:], xt_t[:, ts * P:(ts + 1) * P],
                                wv_sb[:, mb, :], start=st, stop=sp)
                        for hl in range(2):
                            nc.tensor.matmul(
                                psq[hl][:], wq_sb[:, mb, hl * P:(hl + 1) * P],
                                xt_t[:], start=st, stop=sp)
                            nc.tensor.matmul(
                                psk[hl][:], wk_sb[:, mb, hl * P:(hl + 1) * P],
                                xt_t[:], start=st, stop=sp)

                    for ts in range(4):
                        tb = tcx * 4 + ts
                        nc.scalar.activation(
                            v_sb[:, tb * 256:(tb + 1) * 256], psv[ts][:], AFT.Copy)

                    # RoPE: rot_even = qe*cos - qo*sin ; rot_odd = qe*sin + qo*cos
                    # Phase 1 per psum bank: one full-width DVE product against
                    # the duplicated-cos tile (pab = [qe*cos ; qo*cos]) plus two
                    # small PSUM->SBUF DMA partition swaps (sh = [qo ; qe]).
                    # This releases the projection PSUM bank after ~1us so the
                    # next chunk's matmuls start immediately.  Phase 2 (DMA-
                    # gated, off the critical path): pcd = sh * sin_dup =
                    # [qo*sin ; qe*sin], then same-base combines.
                    groups = []
                    for hl in range(2):
                        groups.append((psq[hl], qT_sb[hl]))
                        groups.append((psk[hl], kT_sb[hl]))
                    phase2 = []
                    for ps, dst in groups:
                        pab = rtmp.tile([P, NQ], F32, tag="pab", name="pab",
                                        bufs=3)
                        nc.vector.tensor_mul(pab[:], ps[:], cc_sb[:, tsl])
                        sh = rtmp.tile([P, NQ], F32, tag="sh", name="sh",
                                       bufs=2)
                        # partition-half swap via cross-base copies off a PSUM
                        # operand (ScalarE, off the DVE critical path)
                        nc.scalar.activation(sh[0:64, :], ps[64:128, :], AFT.Copy)
                        nc.scalar.activation(sh[64:128, :], ps[0:64, :], AFT.Copy)
                        phase2.append((pab, sh, dst))
                    for pab, sh, dst in phase2:
                        pcd = rtmp.tile([P, NQ], F32, tag="pcd", name="pcd", bufs=1)
                        nc.vector.tensor_mul(pcd[:], sh[:], ss_sb[:, tsl])
                        nc.vector.tensor_sub(
                            dst[0:64, tsl], pab[0:64, :], pcd[0:64, :])
                        nc.vector.tensor_add(
                            dst[64:128, tsl], pcd[64:128, :], pab[64:128, :])


                # ---- attention for the two heads of this pair ----
                for hl in range(2):
                    h4 = pair * 2 + hl
                    for qc in range(TC):
                        ps_oT = psum.tile([P, NQ], F32,
                                          tag=("v0", "k0")[qc % 2], name="ps_oT")
                        ps_den = psum.tile([P, NQ], F32,
                                           tag=("v1", "k1")[qc % 2], name="ps_den")
                        jmax = 4 * qc + 3
                        for j in range(jmax + 1):
                            pat = j - 4 * qc
                            q0 = 128 * pat if pat >= 0 else 0
                            qs = slice(qc * NQ + q0, (qc + 1) * NQ)
                            st, sp = (j == 0), (j == jmax)
                            ps_sT = psum.tile([P, NQ], F32,
                                              tag=("q0", "q1", "v2")[j % 3],
                                              name="ps_sT")
                            nc.tensor.matmul(
                                ps_sT[:, q0:NQ],
                                kT_sb[hl][:, j * P:(j + 1) * P],
                                qT_sb[hl][:, qs], start=True, stop=True)
                            pT = ppool.tile([P, NQ], F32R, tag="pT", name="pT")
                            nc.scalar.activation(
                                pT[:, q0:NQ], ps_sT[:, q0:NQ], AFT.Exp)
                            if pat >= 0:
                                nc.vector.tensor_mul(
                                    pT[:, q0:NQ],
                                    pT[:, q0:NQ].bitcast(F32),
                                    mask_sb[:, pat * NQ + q0:(pat + 1) * NQ])
                            nc.tensor.matmul(
                                ps_oT[:, q0:NQ],
                                v_sb[:, j * 256 + hl * P: j * 256 + hl * P + P],
                                pT[:, q0:NQ], start=st, stop=sp)
                            nc.tensor.matmul(
                                ps_den[:, q0:NQ], ones_sb[:],
                                pT[:, q0:NQ], start=st, stop=sp)
                        rec = rtmp.tile([P, NQ], F32, tag="rec", name="rec", bufs=1)
                        nc.vector.reciprocal(rec[:], ps_den[:])
                        nc.vector.tensor_mul(
                            oT_sb[:, h4 * T + qc * NQ: h4 * T + (qc + 1) * NQ],
                            ps_oT[:], rec[:])

            # ---- output projection: r[t, m] = sum_h oT_h.T @ wo_h ----
            for mc in range(4):
                wo_mc = wopool.tile([P, HPG, NQ], F32R, tag="womc", name="wo_mc")
                nc.sync.dma_start(out=wo_mc[:], in_=wo_d[:, :, mc * NQ:(mc + 1) * NQ])
                for tb in range(TB):
                    ps_r = psum.tile([P, NQ], F32, tag=("q0", "q1", "v2")[tb % 3], name="ps_r")
                    for h4 in range(HPG):
                        nc.tensor.matmul(
                            ps_r[:],
                            oT_sb[:, h4 * T + tb * P: h4 * T + (tb + 1) * P],
                            wo_mc[:, h4, :],
                            start=(h4 == 0), stop=(h4 == HPG - 1))
                    ro = rout.tile([P, NQ], F32, tag="ro", name="ro")
                    nc.scalar.activation(ro[:], ps_r[:], AFT.Copy)
                    # store on the gpsimd queue so wo prefetch (sync queue)
                    # is not blocked behind 16 result stores
                    nc.gpsimd.dma_start(
                        out=r_d[tb * P:(tb + 1) * P, mc * NQ:(mc + 1) * NQ],
                        in_=ro[:])

    nc.compile()
    return nc


def _host_constants():
    half = D // 2
    pos = np.arange(T, dtype=np.float32)[:, None]
    freqs = np.power(
        np.float32(ROTARY_BASE),
        -np.arange(half, dtype=np.float32) / np.float32(half))[None, :]
    rad = pos * freqs                              # [T, 64]
    cos = np.cos(rad).astype(np.float32).T         # [64, T]
    sin = np.sin(rad).astype(np.float32).T         # [64, T]
    cc = np.concatenate([cos, cos], axis=0)        # cos duplicated on both halves
    ss = np.concatenate([sin, sin], axis=0)        # sin duplicated

    # mask pattern p: allowed (1.0) where 128*p + kk <= qq
    kk = np.arange(P)[:, None]
    qq = np.arange(NQ)[None, :]
    mask = np.concatenate(
        [(P * p + kk <= qq).astype(np.float32) for p in range(4)], axis=1)

    ones = np.ones((P, P), dtype=np.float32)
    return cc, ss, mask, ones


def kernel(x, wq, wk, wv, wo):
    x = np.ascontiguousarray(np.asarray(x, dtype=np.float32))
    wq = np.asarray(wq, dtype=np.float32)
    wk = np.asarray(wk, dtype=np.float32)
    wv = np.asarray(wv, dtype=np.float32)
    wo = np.asarray(wo, dtype=np.float32)

    from concourse.bass_utils import run_bass_kernel_spmd

    if "nc" not in _CACHE:
        _CACHE["nc"] = _build_program()
    nc = _CACHE["nc"]

    cc, ss, mask, ones = _host_constants()
    mult = np.float32(np.sqrt(ATTN_SCALE))

    def w_layout(w, g, scale):
        # w: [M, H, D] -> group slice -> [P, 2, MB, 256]
        ws = (w[:, g * HPG:(g + 1) * HPG, :] * scale).astype(np.float32)
        ws = ws.reshape(M, 2, 256)                    # pair-major head axis
        ws = ws.reshape(MB, P, 2, 256).transpose(1, 2, 0, 3)
        return np.ascontiguousarray(ws)

    in_maps = []
    for c in range(N_CORES):
        b, g = divmod(c, GROUPS)
        xt = np.ascontiguousarray(x[b].T)            # [M, T]
        wog = np.ascontiguousarray(
            wo[g * HPG:(g + 1) * HPG].transpose(1, 0, 2))  # [D, HPG, M]
        in_maps.append({
            "xt": xt,
            "wq": w_layout(wq, g, mult),
            "wk": w_layout(wk, g, mult),
            "wv": w_layout(wv, g, np.float32(1.0)),
            "wo": wog,
            "trig_cc": cc,
            "trig_ss": ss,
            "mask01": mask,
            "onesw": ones,
        })

    res = run_bass_kernel_spmd(nc, in_maps, list(range(N_CORES)))

    r = np.zeros((B, T, M), dtype=np.float32)
    for c in range(N_CORES):
        b = c // GROUPS
        r[b] += res.results[c]["r_out"]
    return r


# revision 27
# speedup vs baseline: 1.0198x; 1.0198x over previous
"""Multi-head causal attention (RoPE, muP scale) on 8 TRN2 NeuronCores.

Sharding: core c = (b, g) with b = c // 4 (data-parallel batch), g = c % 4
(tensor-parallel head group of 4 heads).  Each core computes q/k/v
projections for its 4 heads, RoPE, causal flash-style attention in the
transposed (sT = [k, q]) orientation, and a partial output projection
o @ wo over its heads.  The host sums the 4 per-group partials per batch
(the tensor-parallel reduce) and stacks the 2 batches.

Matmuls run as float32r (FP22 reads, 1.5 cyc/row) accumulating in fp32
PSUM; softmax runs without max-subtraction (scores are O(0.1) by muP
scaling so exp is well-conditioned), with the causal mask applied as a
0/1 multiply on the diagonal blocks and the denominator accumulated by
an all-ones matmul into a broadcast PSUM tile.
"""

import sys

if "/opt/trn_rl_repo" not in sys.path:
    sys.path.insert(0, "/opt/trn_rl_repo")

import numpy as np

B, T, M, H, D = 2, 2048, 2048, 16, 128
N_CORES = 8
GROUPS = 4
HPG = H // GROUPS          # heads per group (4)
ROTARY_BASE = 10000.0
ATTN_SCALE = 1.0 / 128.0

P = 128                    # partitions
TC = T // 512              # 4 t-chunks of 512
MB = M // P                # 16 m-blocks
TB = T // P                # 16 t-blocks
NQ = 512                   # q-chunk width

_CACHE = {}


def _build_program():
    from concourse import bacc, tile
    import concourse.mybir as mybir

    F32 = mybir.dt.float32
    F32R = mybir.dt.float32r
    AFT = mybir.ActivationFunctionType

    nc = bacc.Bacc("TRN2", target_bir_lowering=False, debug=False,
                   num_devices=N_CORES)

    xt_d = nc.dram_tensor("xt", [M, T], F32R, kind="ExternalInput")
    wq_d = nc.dram_tensor("wq", [P, 2, MB, 256], F32R, kind="ExternalInput")
    wk_d = nc.dram_tensor("wk", [P, 2, MB, 256], F32R, kind="ExternalInput")
    wv_d = nc.dram_tensor("wv", [P, 2, MB, 256], F32R, kind="ExternalInput")
    wo_d = nc.dram_tensor("wo", [P, HPG, M], F32R, kind="ExternalInput")
    cc_d = nc.dram_tensor("trig_cc", [P, T], F32, kind="ExternalInput")
    ss_d = nc.dram_tensor("trig_ss", [P, T], F32, kind="ExternalInput")
    mask_d = nc.dram_tensor("mask01", [P, 4 * NQ], F32, kind="ExternalInput")
    ones_d = nc.dram_tensor("onesw", [P, P], F32R, kind="ExternalInput")
    r_d = nc.dram_tensor("r_out", [T, M], F32, kind="ExternalOutput")

    with tile.TileContext(nc) as tc:
        with (
            tc.tile_pool(name="consts", bufs=1) as consts,
            tc.tile_pool(name="wpool", bufs=1) as wpool,
            tc.tile_pool(name="wopool", bufs=2) as wopool,
            tc.tile_pool(name="qkv", bufs=1) as qkv,
            tc.tile_pool(name="xpool", bufs=6) as xpool,
            tc.tile_pool(name="ppool", bufs=2) as ppool,
            tc.tile_pool(name="rtmp", bufs=2) as rtmp,
            tc.tile_pool(name="opool", bufs=1) as opool,
            tc.tile_pool(name="rout", bufs=2) as rout,
            tc.tile_pool(name="psum", bufs=1, space="PSUM") as psum,
        ):
            cc_sb = consts.tile([P, T], F32, tag="cc")
            nc.sync.dma_start(out=cc_sb[:], in_=cc_d[:])
            ss_sb = consts.tile([P, T], F32, tag="ss")
            nc.sync.dma_start(out=ss_sb[:], in_=ss_d[:])
            mask_sb = consts.tile([P, 4 * NQ], F32, tag="mask")
            nc.sync.dma_start(out=mask_sb[:], in_=mask_d[:])
            ones_sb = consts.tile([P, P], F32R, tag="ones")
            nc.sync.dma_start(out=ones_sb[:], in_=ones_d[:])

            # oT for all 4 heads of the group: [d, h4 * T + t]
            oT_sb = opool.tile([P, HPG * T], F32R, tag="oT")

            xt_tiles = {}

            for pair in range(2):
                wq_sb = wpool.tile([P, MB, 256], F32R, tag="wq", name="wq_sb")
                nc.sync.dma_start(out=wq_sb[:], in_=wq_d[:, pair])
                wk_sb = wpool.tile([P, MB, 256], F32R, tag="wk", name="wk_sb")
                nc.sync.dma_start(out=wk_sb[:], in_=wk_d[:, pair])
                wv_sb = wpool.tile([P, MB, 256], F32R, tag="wv", name="wv_sb")
                nc.sync.dma_start(out=wv_sb[:], in_=wv_d[:, pair])

                qT_sb = [qkv.tile([P, T], F32R, tag=f"qT{hl}", name=f"qT{hl}")
                         for hl in range(2)]
                kT_sb = [qkv.tile([P, T], F32R, tag=f"kT{hl}", name=f"kT{hl}")
                         for hl in range(2)]
                v_sb = qkv.tile([P, TB * 256], F32R, tag="v", name="v_sb")

                # ---- projections + RoPE for this head pair ----
                # xt tile prefetcher: explicit round-robin tags (FIFO slot
                # reuse) and next-chunk DMAs emitted inside the current
                # chunk's tail so the scheduler places them (and their slot
                # waits) early enough to hide the refill at chunk boundaries.
                def ensure_xt(i):
                    if i in xt_tiles:
                        return xt_tiles[i]
                    tcx_i = (i // MB) % TC
                    mb_i = i % MB
                    xt_t = xpool.tile([P, NQ], F32R, bufs=1,
                                      tag=f"xt{i % 8}", name="xt_t")
                    eng = nc.sync if i % 2 == 0 else nc.gpsimd
                    eng.dma_start(
                        out=xt_t[:],
                        in_=xt_d[mb_i * P:(mb_i + 1) * P,
                                 tcx_i * NQ:(tcx_i + 1) * NQ],
                    )
                    xt_tiles[i] = xt_t
                    return xt_t

                for tcx in range(TC):
                    gidx = pair * TC + tcx
                    tsl = slice(tcx * NQ, (tcx + 1) * NQ)
                    psq = [psum.tile([P, NQ], F32, tag=f"q{hl}", name=f"psq{hl}")
                           for hl in range(2)]
                    psk = [psum.tile([P, NQ], F32, tag=f"k{hl}", name=f"psk{hl}")
                           for hl in range(2)]
                    psv = [psum.tile([P, 256], F32, tag=f"v{ts}", name=f"psv{ts}")
                           for ts in range(4)]
                    for mb in range(MB):
                        i = gidx * MB + mb
                        xt_t = ensure_xt(i)
                        xt_tiles.pop(i, None)
                        if mb >= MB - 8 and i + 8 < 2 * TC * MB:
                            ensure_xt(i + 8)
                        st, sp = (mb == 0), (mb == MB - 1)
                        # v first: its PSUM banks free fastest at chunk
                        # boundaries, keeping PE fed while RoPE drains q/k.
                        for ts in range(4):
                            nc.tensor.matmul(
                                psv[ts][:], xt_t[:, ts * P:(ts + 1) * P],
                                wv_sb[:, mb, :], start=st, stop=sp)
                        for hl in range(2):
                            nc.tensor.matmul(
                                psq[hl][:], wq_sb[:, mb, hl * P:(hl + 1) * P],
                                xt_t[:], start=st, stop=sp)
                            nc.tensor.matmul(
                                psk[hl][:], wk_sb[:, mb, hl * P:(hl + 1) * P],
                                xt_t[:], start=st, stop=sp)

                    for ts in range(4):
                        tb = tcx * 4 + ts
                        nc.scalar.activation(
                            v_sb[:, tb * 256:(tb + 1) * 256], psv[ts][:], AFT.Copy)

                    # RoPE: rot_even = qe*cos - qo*sin ; rot_odd = qe*sin + qo*cos
                    # Phase 1 per psum bank: one full-width DVE product against
                    # the duplicated-cos tile (pab = [qe*cos ; qo*cos]) plus a
                    # partition-half swap done as two cross-base ScalarE copies
                    # off the PSUM operand (sh = [qo ; qe]).  This releases the
                    # projection PSUM bank quickly so the next chunk's matmuls
                    # start immediately.  Phase 2 (off the critical path):
                    # pcd = sh * sin_dup = [qo*sin ; qe*sin], then same-base
                    # DVE combines.
                    groups = []
                    for hl in range(2):
                        groups.append((psq[hl], qT_sb[hl]))
                        groups.append((psk[hl], kT_sb[hl]))
                    phase2 = []
                    for ps, dst in groups:
                        pab = rtmp.tile([P, NQ], F32, tag="pab", name="pab",
                                        bufs=3)
                        nc.vector.tensor_mul(pab[:], ps[:], cc_sb[:, tsl])
                        sh = rtmp.tile([P, NQ], F32, tag="sh", name="sh",
                                       bufs=2)
                        # partition-half swap via cross-base copies off a PSUM
                        # operand (ScalarE, off the DVE critical path)
                        nc.scalar.activation(sh[0:64, :], ps[64:128, :], AFT.Copy)
                        nc.scalar.activation(sh[64:128, :], ps[0:64, :], AFT.Copy)
                        phase2.append((pab, sh, dst))
                    for pab, sh, dst in phase2:
                        pcd = rtmp.tile([P, NQ], F32, tag="pcd", name="pcd", bufs=1)
                        nc.vector.tensor_mul(pcd[:], sh[:], ss_sb[:, tsl])
                        nc.vector.tensor_sub(
                            dst[0:64, tsl], pab[0:64, :], pcd[0:64, :])
                        nc.vector.tensor_add(
                            dst[64:128, tsl], pcd[64:128, :], pab[64:128, :])


                # ---- attention for the two heads of this pair ----
                for hl in range(2):
                    h4 = pair * 2 + hl
                    for qc in range(TC):
                        ps_oT = psum.tile([P, NQ], F32,
                                          tag=("v0", "k0")[qc % 2], name="ps_oT")
                        ps_den = psum.tile([P, NQ], F32,
                                           tag=("v1", "k1")[qc % 2], name="ps_den")
                        jmax = 4 * qc + 3
                        for j in range(jmax + 1):
                            pat = j - 4 * qc
                            q0 = 128 * pat if pat >= 0 else 0
                            qs = slice(qc * NQ + q0, (qc + 1) * NQ)
                            st, sp = (j == 0), (j == jmax)
                            ps_sT = psum.tile([P, NQ], F32,
                                              tag=("q0", "q1", "v2")[j % 3],
                                              name="ps_sT")
                            nc.tensor.matmul(
                                ps_sT[:, q0:NQ],
                                kT_sb[hl][:, j * P:(j + 1) * P],
                                qT_sb[hl][:, qs], start=True, stop=True)
                            pT = ppool.tile([P, NQ], F32R, tag="pT", name="pT")
                            nc.scalar.activation(
                                pT[:, q0:NQ], ps_sT[:, q0:NQ], AFT.Exp)
                            if pat >= 0:
                                nc.vector.tensor_mul(
                                    pT[:, q0:NQ],
                                    pT[:, q0:NQ].bitcast(F32),
                                    mask_sb[:, pat * NQ + q0:(pat + 1) * NQ])
                            nc.tensor.matmul(
                                ps_oT[:, q0:NQ],
                                v_sb[:, j * 256 + hl * P: j * 256 + hl * P + P],
                                pT[:, q0:NQ], start=st, stop=sp)
                            nc.tensor.matmul(
                                ps_den[:, q0:NQ], ones_sb[:],
                                pT[:, q0:NQ], start=st, stop=sp)
                        rec = rtmp.tile([P, NQ], F32, tag="rec", name="rec", bufs=1)
                        nc.vector.reciprocal(rec[:], ps_den[:])
                        nc.vector.tensor_mul(
                            oT_sb[:, h4 * T + qc * NQ: h4 * T + (qc + 1) * NQ],
                            ps_oT[:], rec[:])

            # ---- output projection: r[t, m] = sum_h oT_h.T @ wo_h ----
            for mc in range(4):
                wo_mc = wopool.tile([P, HPG, NQ], F32R, tag="womc", name="wo_mc")
                nc.sync.dma_start(out=wo_mc[:], in_=wo_d[:, :, mc * NQ:(mc + 1) * NQ])
                for tb in range(TB):
                    ps_r = psum.tile([P, NQ], F32, tag=("q0", "q1", "v2")[tb % 3], name="ps_r")
                    for h4 in range(HPG):
                        nc.tensor.matmul(
                            ps_r[:],
                            oT_sb[:, h4 * T + tb * P: h4 * T + (tb + 1) * P],
                            wo_mc[:, h4, :],
                            start=(h4 == 0), stop=(h4 == HPG - 1))
                    ro = rout.tile([P, NQ], F32, tag="ro", name="ro")
                    nc.scalar.activation(ro[:], ps_r[:], AFT.Copy)
                    # store on the gpsimd queue so wo prefetch (sync queue)
                    # is not blocked behind 16 result stores
                    nc.gpsimd.dma_start(
                        out=r_d[tb * P:(tb + 1) * P, mc * NQ:(mc + 1) * NQ],
                        in_=ro[:])

    nc.compile()
    return nc


def _host_constants():
    half = D // 2
    pos = np.arange(T, dtype=np.float32)[:, None]
    freqs = np.power(
        np.float32(ROTARY_BASE),
        -np.arange(half, dtype=np.float32) / np.float32(half))[None, :]
    rad = pos * freqs                              # [T, 64]
    cos = np.cos(rad).astype(np.float32).T         # [64, T]
    sin = np.sin(rad).astype(np.float32).T         # [64, T]
    cc = np.concatenate([cos, cos], axis=0)        # cos duplicated on both halves
    ss = np.concatenate([sin, sin], axis=0)        # sin duplicated

    # mask pattern p: allowed (1.0) where 128*p + kk <= qq
    kk = np.arange(P)[:, None]
    qq = np.arange(NQ)[None, :]
    mask = np.concatenate(
        [(P * p + kk <= qq).astype(np.float32) for p in range(4)], axis=1)

    ones = np.ones((P, P), dtype=np.float32)
    return cc, ss, mask, ones


def kernel(x, wq, wk, wv, wo):
    x = np.ascontiguousarray(np.asarray(x, dtype=np.float32))
    wq = np.asarray(wq, dtype=np.float32)
    wk = np.asarray(wk, dtype=np.float32)
    wv = np.asarray(wv, dtype=np.float32)
    wo = np.asarray(wo, dtype=np.float32)

    from concourse.bass_utils import run_bass_kernel_spmd

    if "nc" not in _CACHE:
        _CACHE["nc"] = _build_program()
    nc = _CACHE["nc"]

    cc, ss, mask, ones = _host_constants()
    mult = np.float32(np.sqrt(ATTN_SCALE))

    def w_layout(w, g, scale):
        # w: [M, H, D] -> group slice -> [P, 2, MB, 256]
        ws = (w[:, g * HPG:(g + 1) * HPG, :] * scale).astype(np.float32)
        ws = ws.reshape(M, 2, 256)                    # pair-major head axis
        ws = ws.reshape(MB, P, 2, 256).transpose(1, 2, 0, 3)
        return np.ascontiguousarray(ws)

    in_maps = []
    for c in range(N_CORES):
        b, g = divmod(c, GROUPS)
        xt = np.ascontiguousarray(x[b].T)            # [M, T]
        wog = np.ascontiguousarray(
            wo[g * HPG:(g + 1) * HPG].transpose(1, 0, 2))  # [D, HPG, M]
        in_maps.append({
            "xt": xt,
            "wq": w_layout(wq, g, mult),
            "wk": w_layout(wk, g, mult),
            "wv": w_layout(wv, g, np.float32(1.0)),
            "wo": wog,
            "trig_cc": cc,
            "trig_ss": ss,
            "mask01": mask,
            "onesw": ones,
        })

    res = run_bass_kernel_spmd(nc, in_maps, list(range(N_CORES)))

    r = np.zeros((B, T, M), dtype=np.float32)
    for c in range(N_CORES):
        b = c // GROUPS
        r[b] += res.results[c]["r_out"]
    return r
